# revision 62
# baseline (speedup 1.0000x reference)
import sys
import numpy as np

sys.path.insert(0, '/opt/trn_rl_repo')

NH, DK, DV, FILTERS = 8, 64, 64, 128
B, C, H, W = 4, 64, 32, 32
HW = H * W
dkh = DK // NH
SCALE = dkh ** -0.5
N_CORES = 8

XW_X = 0
XW_WKQ = XW_X + HW
XW_WPAT = XW_WKQ + 256
XW_WVA = XW_WPAT + 504
XW_WCONV = XW_WVA + 128
XW_XC = XW_WCONV + 64
XW_COLS = XW_XC + 512
XW_CUT = XW_WPAT + 504


def _build_bass(debug=False):
    import concourse.bass as bass
    import concourse.bacc as bacc
    import concourse.mybir as mybir
    import concourse.tile as tile

    f32 = mybir.dt.float32
    bf16 = mybir.dt.bfloat16
    AF = mybir.ActivationFunctionType

    nc = bacc.Bacc()

    xw = nc.dram_tensor("xw", [65, XW_COLS], bf16, kind="ExternalInput")
    den_scr = nc.dram_tensor("den_scr", [4, HW], bf16)
    kfull = nc.dram_tensor("kfull", [128, HW], bf16, kind="ExternalInput")
    wtail = nc.dram_tensor("wtail", [128, 96], bf16, kind="ExternalInput")
    o_conv = nc.dram_tensor("o_conv", [64, 512], f32, kind="ExternalOutput")
    o_attn = nc.dram_tensor("o_attn", [64, HW], f32, kind="ExternalOutput")
    if debug:
        d_qa0 = nc.dram_tensor("d_qa0", [128, HW], bf16, kind="ExternalOutput")
        d_pt00 = nc.dram_tensor("d_pt00", [128, HW], bf16, kind="ExternalOutput")

    with tile.TileContext(nc) as tc:
        with (
            tc.tile_pool(name="const", bufs=1) as constp,
            tc.tile_pool(name="proj", bufs=1) as projp,
            tc.tile_pool(name="kaqa", bufs=1) as kaqap,
            tc.tile_pool(name="pt", bufs=8) as ptp,
            tc.tile_pool(name="tail", bufs=1) as tailp,
            tc.tile_pool(name="pslg", bufs=2, space="PSUM") as pslg,
            tc.tile_pool(name="psmisc", bufs=1, space="PSUM") as psmisc,
            tc.tile_pool(name="pspv", bufs=1, space="PSUM") as pspv,
        ):
            xw_sb = constp.tile([65, XW_COLS], bf16, tag="xw")
            nc.sync.dma_start(out=xw_sb[:, 0:XW_CUT], in_=xw[:, 0:XW_CUT])
            nc.scalar.dma_start(out=xw_sb[:, XW_CUT:XW_COLS],
                                in_=xw[:, XW_CUT:XW_COLS])
            x_bf = xw_sb[:, XW_X:XW_X + HW]
            wva_sb = xw_sb[:, XW_WVA:XW_WVA + 128]
            wpat_sb = xw_sb[:, XW_WPAT:XW_WPAT + 504].rearrange(
                "c (h m) -> c h m", h=4)
            wconv_sb = xw_sb[:, XW_WCONV:XW_WCONV + 64]
            wkq_sb = xw_sb[:, XW_WKQ:XW_WKQ + 256].rearrange(
                "c (h m) -> c h m", h=4)
            xc_sb = xw_sb[:, XW_XC:XW_XC + 512]

            kfull_sb = constp.tile([128, HW], bf16, tag="kfull")
            nc.scalar.dma_start(out=kfull_sb, in_=kfull[:, :])
            wtail_sb = constp.tile([128, 96], bf16, tag="wtail")
            nc.sync.dma_start(out=wtail_sb, in_=wtail[:, :])
            wattn_sb = wtail_sb[:, 0:64]
            ones_sb = wtail_sb[:, 64:96]

            warm_sb = projp.tile([128, 512], bf16, tag="warm")
            nc.gpsimd.memset(warm_sb, 0.0)
            warm_ps = psmisc.tile([64, 512], f32, tag="misc", name="warm_ps")
            for _ in range(4):
                nc.tensor.matmul(warm_ps[:, :], warm_sb[0:65, 0:64],
                                 warm_sb[0:65, :])

            QA = [None] * 4

            def head_prep_chunks(i):
                state = {}

                def c_y(half):
                    if half == 0:
                        state['pat'] = psmisc.tile([128, HW], f32, tag="misc",
                                                   name=f"pat_ps{i}")
                    s = slice(512 * half, 512 * half + 512)
                    nc.tensor.matmul(state['pat'][0:64, s], wkq_sb[:, i, :],
                                     x_bf[:, s], tile_position=(0, 0))

                def c_patw(half):
                    pat_ps = state['pat']
                    xr = x_bf.rearrange("p (r c) -> p r c", r=32)
                    patw = pat_ps[64:96, :].rearrange("p (r c) -> p r c", r=32)
                    rs = slice(16 * half, 16 * half + 16)
                    for c in range(32):
                        nc.tensor.matmul(patw[:, rs, c],
                                         wpat_sb[:, i, 31 - c:63 - c],
                                         xr[:, rs, c],
                                         tile_position=(0, 64))

                def c_path(half):
                    pat_ps = state['pat']
                    for r in range(16 * half, 16 * half + 16):
                        nc.tensor.matmul(
                            pat_ps[96:128, 32 * r:32 * r + 32],
                            wpat_sb[:, i, 94 - r:126 - r],
                            x_bf[:, 32 * r:32 * r + 32],
                            tile_position=(0, 96))

                def c_qa(half):
                    if QA[i] is None:
                        QA[i] = kaqap.tile([128, HW], bf16, tag=f"qa{i}",
                                           name=f"qa{i}")
                    s = slice(512 * half, 512 * half + 512)
                    if i == 0 and half == 0:
                        nc.scalar.activation(QA[i][:, s], state['pat'][:, s],
                                             AF.Copy)
                    else:
                        nc.vector.tensor_copy(out=QA[i][:, s],
                                              in_=state['pat'][:, s])

                return [lambda: c_y(0), lambda: c_y(1),
                        lambda: c_patw(0), lambda: c_path(0),
                        lambda: c_qa(0),
                        lambda: (c_patw(1), c_path(1), c_qa(1))]

            for ch in head_prep_chunks(0):
                ch()

            vt_ps = pspv.tile([128, 8, 128], f32, tag="pv", name="vt_ps")
            for kt in range(8):
                nc.tensor.matmul(vt_ps[:, kt, :],
                                 x_bf[:, 128 * kt:128 * kt + 128],
                                 wva_sb)
            vt_sb = projp.tile([128, 8, 128], bf16, tag="vt")
            nc.vector.tensor_copy(out=vt_sb[:, :, :], in_=vt_ps[:, :, :])

            pv_ps = pspv.tile([128, HW], f32, tag="pv")
            pv_sb = tailp.tile([128, HW], bf16, tag="pv_sb")
            rp = tailp.tile([128, HW], bf16, tag="rp")
            attn_n = tailp.tile([128, HW], bf16, tag="attn_n")
            seq = [(i, kt) for i in range(4) for kt in range(8)]

            def emit_lg(i, kt):
                lg_ps = pslg.tile([128, HW], f32, tag="lg", name=f"lg{i}_{kt}")
                for qc in range(2):
                    nc.tensor.matmul(
                        lg_ps[:, 512 * qc:512 * qc + 512],
                        kfull_sb[:, 128 * kt:128 * kt + 128],
                        QA[i][:, 512 * qc:512 * qc + 512])
                return lg_ps

            def emit_conv():
                conv_ps = psmisc.tile([64, 512], f32, tag="misc")
                nc.tensor.matmul(conv_ps[:, :], wconv_sb, xc_sb)
                conv_sb = projp.tile([64, 512], f32, tag="conv")
                nc.vector.tensor_copy(out=conv_sb, in_=conv_ps[:, :])
                nc.sync.dma_start(out=o_conv[:, :], in_=conv_sb)

            def emit_recip(i, half):
                r0 = 32 * i
                s = slice(512 * half, 512 * half + 512)
                with nc.allow_low_precision(reason="bf16 softmax denom"):
                    nc.vector.reciprocal(out=rp[r0:r0 + 1, s],
                                         in_=pv_sb[r0:r0 + 1, s])

            rpb = tailp.tile([128, HW], bf16, tag="rpb")

            def emit_bounce(i):
                r0 = 32 * i
                nc.gpsimd.dma_start(out=den_scr[i, :], in_=rp[r0:r0 + 1, :])
                rep = bass.AP(den_scr, i * HW, [[0, 32], [1, HW]])
                nc.gpsimd.dma_start(out=rpb[r0:r0 + 32, :], in_=rep)

            def emit_mul(i):
                r0 = 32 * i
                sl = slice(0, HW)
                nc.vector.tensor_mul(attn_n[r0:r0 + 32, sl],
                                     pv_sb[r0:r0 + 32, sl],
                                     rpb[r0:r0 + 32, sl])

            jobs = []
            jobs.extend(head_prep_chunks(1))
            next_prep = 2
            lg_tiles = {seq[0]: emit_lg(*seq[0])}
            for j, (i, kt) in enumerate(seq):
                if kt == 7 and jobs:
                    jobs.pop(0)()
                if j + 1 < len(seq):
                    lg_tiles[seq[j + 1]] = emit_lg(*seq[j + 1])
                lg_ps = lg_tiles.pop((i, kt))
                pt = ptp.tile([128, HW], bf16)
                nc.scalar.activation(pt, lg_ps[:, :], AF.Exp)
                if debug and i == 0 and kt == 0:
                    nc.sync.dma_start(out=d_pt00[:, :], in_=pt)
                for qc in range(2):
                    nc.tensor.matmul(
                        pv_ps[32 * i:32 * i + 32, 512 * qc:512 * qc + 512],
                        vt_sb[:, kt, 32 * i:32 * i + 32],
                        pt[:, 512 * qc:512 * qc + 512],
                        start=(kt == 0), stop=(kt == 7),
                        tile_position=(0, 32 * i))
                if kt != 7 and jobs:
                    jobs.pop(0)()
                if kt == 7:
                    if i < 3:
                        nc.vector.tensor_copy(out=pv_sb[32 * i:32 * i + 32, :],
                                              in_=pv_ps[32 * i:32 * i + 32, :])
                        emit_recip(i, 0)
                        emit_recip(i, 1)
                        emit_bounce(i)
                    if next_prep < 4:
                        jobs.extend(head_prep_chunks(next_prep))
                        next_prep += 1
                    elif i == 2:
                        jobs.append(emit_conv)
                    if i < 3:
                        jobs.append(lambda i=i: emit_mul(i))
            if debug:
                nc.sync.dma_start(out=d_qa0[:, :], in_=QA[0])

            rpb = tailp.tile([128, HW], bf16, tag="rpb3", name="rpb3")
            oat_ps = pslg.tile([64, HW], f32, tag="lg")
            oat_sb = tailp.tile([64, HW], f32, tag="oat")
            HS = [slice(0, 512), slice(512, 1024)]
            recb3 = [None, None]
            with nc.allow_low_precision(reason="bf16 softmax denom"):
                for h, s in enumerate(HS):
                    nc.vector.reciprocal(out=rp[96:97, s], in_=pv_ps[96:97, s])
            for h, s in enumerate(HS):
                pool, tag = (psmisc, "misc") if h == 0 else (pslg, "lg")
                recb3[h] = pool.tile([128, HW], f32, tag=tag,
                                     name=f"recb3_{h}")
                nc.tensor.matmul(recb3[h][96:128, s], ones_sb[96:97, :],
                                 rp[96:97, s], tile_position=(96, 96))
                nc.scalar.activation(rpb[96:128, s], recb3[h][96:128, s],
                                     AF.Copy)
            for h, s in enumerate(HS):
                nc.vector.tensor_mul(attn_n[96:128, s], pv_ps[96:128, s],
                                     rpb[96:128, s])
            for h, s in enumerate(HS):
                nc.tensor.matmul(oat_ps[:, s], wattn_sb, attn_n[:, s])
            nc.vector.tensor_copy(out=oat_sb[:, HS[0]], in_=oat_ps[:, HS[0]])
            nc.sync.dma_start(out=o_attn[:, HS[0]], in_=oat_sb[:, HS[0]])
            nc.vector.tensor_copy(out=oat_sb[:, HS[1]], in_=oat_ps[:, HS[1]])
            nc.scalar.dma_start(out=o_attn[:, HS[1]], in_=oat_sb[:, HS[1]])

    nc.compile()
    return nc


def _host_prep(inputs):
    import ml_dtypes
    bf = ml_dtypes.bfloat16
    x = np.ascontiguousarray(inputs['x'], np.float32)
    w_qkv = np.ascontiguousarray(inputs['w_qkv'].reshape(2 * DK + DV, C), np.float32)
    b_qkv = np.ascontiguousarray(inputs['b_qkv'], np.float32)
    w_conv = np.ascontiguousarray(inputs['w_conv'].reshape(FILTERS - DV, C), np.float32)
    b_conv = np.ascontiguousarray(inputs['b_conv'], np.float32)
    w_attn = np.ascontiguousarray(inputs['w_attn'].reshape(DV, DV), np.float32)
    b_attn = np.ascontiguousarray(inputs['b_attn'], np.float32)
    rel_h = np.ascontiguousarray(inputs['key_rel_h'], np.float32)
    rel_w = np.ascontiguousarray(inputs['key_rel_w'], np.float32)
    relcat = np.concatenate([rel_w, rel_h], 0)

    kk = np.arange(HW)
    DCmat = np.zeros((64, HW), np.float32)
    DCmat[:32] = (kk[None, :] % 32 == np.arange(32)[:, None])
    DCmat[32:] = (kk[None, :] // 32 == np.arange(32)[:, None])

    wconv_aug = np.ascontiguousarray(
        np.concatenate([w_conv, b_conv[:, None]], 1).T)

    in_maps = []
    for cidx in range(N_CORES):
        b, g = cidx // 2, cidx % 2
        heads = [4 * g + i for i in range(4)]
        x_aug = np.concatenate([x[b].reshape(C, HW),
                                np.ones((1, HW), np.float32)], 0)
        kfull_m = np.concatenate([x[b].reshape(C, HW), DCmat], 0)
        wva_m = np.zeros((65, 4, 32), np.float32)
        wpat_m = np.zeros((65, 4, 126), np.float32)
        wkq_m = np.zeros((65, 4, 64), np.float32)
        for i, h in enumerate(heads):
            wv = w_qkv[128 + 8 * h:128 + 8 * h + 8]
            bv = b_qkv[128 + 8 * h:128 + 8 * h + 8]
            wva_m[64, i, 0] = 1.0
            wva_m[:64, i, 1:9] = wv.T
            wva_m[64, i, 1:9] = bv
            wq_h = w_qkv[8 * h:8 * h + 8] * SCALE
            bq_h = b_qkv[8 * h:8 * h + 8] * SCALE
            wk_h = w_qkv[64 + 8 * h:64 + 8 * h + 8]
            wpat_m[:64, i, :] = (relcat @ wq_h).T
            wpat_m[64, i, :] = relcat @ bq_h
            wq_aug = np.concatenate([wq_h, bq_h[:, None]], 1)
            wkq_m[:, i, :] = (wk_h.T @ wq_aug).T
        wattn_aug = np.zeros((128, 96), np.float32)
        wattn_aug[[0, 32, 64, 96], 64:96] = 1.0
        for i, h in enumerate(heads):
            wattn_aug[32 * i + 1:32 * i + 9, 0:64] = w_attn[:, 8 * h:8 * h + 8].T
        if g == 0:
            wattn_aug[0, 0:64] += b_attn
        xw_m = np.concatenate(
            [x_aug, wkq_m.reshape(65, 256), wpat_m.reshape(65, 504),
             wva_m.reshape(65, 128), wconv_aug,
             x_aug[:, 512 * g:512 * g + 512]], 1)
        assert xw_m.shape[1] == XW_COLS
        in_maps.append({
            'xw': np.ascontiguousarray(xw_m.astype(bf)),
            'kfull': np.ascontiguousarray(kfull_m.astype(bf)),
            'wtail': np.ascontiguousarray(wattn_aug.astype(bf)),
        })
    return in_maps


_CACHED = {}


def kernel(**inputs):
    from concourse.bass_utils import run_bass_kernel_spmd
    if 'nc' not in _CACHED:
        _CACHED['nc'] = _build_bass()
    nc = _CACHED['nc']
    in_maps = _host_prep(inputs)
    res = run_bass_kernel_spmd(nc, in_maps, core_ids=list(range(N_CORES)))
    out = np.zeros((B, FILTERS, HW), np.float32)
    for c in range(N_CORES):
        b, g = c // 2, c % 2
        out[b, :64, 512 * g:512 * g + 512] = res.results[c]['o_conv']
        out[b, 64:] += res.results[c]['o_attn']
    return out.reshape(B, FILTERS, H, W)


# revision 63
# speedup vs baseline: 1.0188x; 1.0188x over previous
import sys
import numpy as np

sys.path.insert(0, '/opt/trn_rl_repo')

NH, DK, DV, FILTERS = 8, 64, 64, 128
B, C, H, W = 4, 64, 32, 32
HW = H * W
dkh = DK // NH
SCALE = dkh ** -0.5
N_CORES = 8

XW_X = 0
XW_WKQ = XW_X + HW
XW_WPAT = XW_WKQ + 256
XW_WVA = XW_WPAT + 504
XW_WCONV = XW_WVA + 128
XW_XC = XW_WCONV + 64
XW_COLS = XW_XC + 512
XW_CUT = XW_WPAT + 504


def _build_bass(debug=False):
    import concourse.bass as bass
    import concourse.bacc as bacc
    import concourse.mybir as mybir
    import concourse.tile as tile

    f32 = mybir.dt.float32
    bf16 = mybir.dt.bfloat16
    AF = mybir.ActivationFunctionType

    nc = bacc.Bacc()

    xw = nc.dram_tensor("xw", [65, XW_COLS], bf16, kind="ExternalInput")
    den_scr = nc.dram_tensor("den_scr", [4, HW], bf16)
    kfull = nc.dram_tensor("kfull", [128, HW], bf16, kind="ExternalInput")
    wtail = nc.dram_tensor("wtail", [128, 96], bf16, kind="ExternalInput")
    o_conv = nc.dram_tensor("o_conv", [64, 512], f32, kind="ExternalOutput")
    o_attn = nc.dram_tensor("o_attn", [64, HW], f32, kind="ExternalOutput")
    if debug:
        d_qa0 = nc.dram_tensor("d_qa0", [128, HW], bf16, kind="ExternalOutput")
        d_pt00 = nc.dram_tensor("d_pt00", [128, HW], bf16, kind="ExternalOutput")

    with tile.TileContext(nc) as tc:
        with (
            tc.tile_pool(name="const", bufs=1) as constp,
            tc.tile_pool(name="proj", bufs=1) as projp,
            tc.tile_pool(name="kaqa", bufs=1) as kaqap,
            tc.tile_pool(name="pt", bufs=8) as ptp,
            tc.tile_pool(name="tail", bufs=1) as tailp,
            tc.tile_pool(name="pslg", bufs=2, space="PSUM") as pslg,
            tc.tile_pool(name="psmisc", bufs=1, space="PSUM") as psmisc,
            tc.tile_pool(name="pspv", bufs=1, space="PSUM") as pspv,
        ):
            xw_sb = constp.tile([65, XW_COLS], bf16, tag="xw")
            nc.sync.dma_start(out=xw_sb[:, 0:XW_CUT], in_=xw[:, 0:XW_CUT])
            nc.scalar.dma_start(out=xw_sb[:, XW_CUT:XW_COLS],
                                in_=xw[:, XW_CUT:XW_COLS])
            x_bf = xw_sb[:, XW_X:XW_X + HW]
            wva_sb = xw_sb[:, XW_WVA:XW_WVA + 128]
            wpat_sb = xw_sb[:, XW_WPAT:XW_WPAT + 504].rearrange(
                "c (h m) -> c h m", h=4)
            wconv_sb = xw_sb[:, XW_WCONV:XW_WCONV + 64]
            wkq_sb = xw_sb[:, XW_WKQ:XW_WKQ + 256].rearrange(
                "c (h m) -> c h m", h=4)
            xc_sb = xw_sb[:, XW_XC:XW_XC + 512]

            kfull_sb = constp.tile([128, HW], bf16, tag="kfull")
            nc.scalar.dma_start(out=kfull_sb, in_=kfull[:, :])
            wtail_sb = constp.tile([128, 96], bf16, tag="wtail")
            nc.sync.dma_start(out=wtail_sb, in_=wtail[:, :])
            wattn_sb = wtail_sb[:, 0:64]
            ones_sb = wtail_sb[:, 64:96]

            warm_sb = projp.tile([128, 512], bf16, tag="warm")
            nc.gpsimd.memset(warm_sb, 0.0)
            warm_ps = psmisc.tile([64, 512], f32, tag="misc", name="warm_ps")
            for _ in range(4):
                nc.tensor.matmul(warm_ps[:, :], warm_sb[0:65, 0:64],
                                 warm_sb[0:65, :])

            QA = [None] * 4

            def head_prep_chunks(i):
                state = {}

                def c_y(half):
                    if half == 0:
                        state['pat'] = psmisc.tile([128, HW], f32, tag="misc",
                                                   name=f"pat_ps{i}")
                    s = slice(512 * half, 512 * half + 512)
                    nc.tensor.matmul(state['pat'][0:64, s], wkq_sb[:, i, :],
                                     x_bf[:, s], tile_position=(0, 0))

                def c_patw(half):
                    pat_ps = state['pat']
                    xr = x_bf.rearrange("p (r c) -> p r c", r=32)
                    patw = pat_ps[64:96, :].rearrange("p (r c) -> p r c", r=32)
                    rs = slice(16 * half, 16 * half + 16)
                    for c in range(32):
                        nc.tensor.matmul(patw[:, rs, c],
                                         wpat_sb[:, i, 31 - c:63 - c],
                                         xr[:, rs, c],
                                         tile_position=(0, 64))

                def c_path(half):
                    pat_ps = state['pat']
                    for r in range(16 * half, 16 * half + 16):
                        nc.tensor.matmul(
                            pat_ps[96:128, 32 * r:32 * r + 32],
                            wpat_sb[:, i, 94 - r:126 - r],
                            x_bf[:, 32 * r:32 * r + 32],
                            tile_position=(0, 96))

                def c_qa(half):
                    if QA[i] is None:
                        QA[i] = kaqap.tile([128, HW], bf16, tag=f"qa{i}",
                                           name=f"qa{i}")
                    s = slice(512 * half, 512 * half + 512)
                    if i == 0 and half == 0:
                        nc.scalar.activation(QA[i][:, s], state['pat'][:, s],
                                             AF.Copy)
                    else:
                        nc.vector.tensor_copy(out=QA[i][:, s],
                                              in_=state['pat'][:, s])

                return [lambda: c_y(0), lambda: c_y(1),
                        lambda: c_patw(0), lambda: c_path(0),
                        lambda: c_qa(0),
                        lambda: (c_patw(1), c_path(1), c_qa(1))]

            for ch in head_prep_chunks(0):
                ch()

            vt_ps = pspv.tile([128, 8, 128], f32, tag="pv", name="vt_ps")
            for kt in range(8):
                nc.tensor.matmul(vt_ps[:, kt, :],
                                 x_bf[:, 128 * kt:128 * kt + 128],
                                 wva_sb)
            vt_sb = projp.tile([128, 8, 128], bf16, tag="vt")
            nc.vector.tensor_copy(out=vt_sb[:, :, :], in_=vt_ps[:, :, :])

            pv_ps = pspv.tile([128, HW], f32, tag="pv")
            pv_sb = tailp.tile([128, HW], bf16, tag="pv_sb")
            rp = tailp.tile([128, HW], bf16, tag="rp")
            attn_n = tailp.tile([128, HW], bf16, tag="attn_n")
            seq = [(i, kt) for i in range(4) for kt in range(8)]

            def emit_lg(i, kt):
                lg_ps = pslg.tile([128, HW], f32, tag="lg", name=f"lg{i}_{kt}")
                for qc in range(2):
                    nc.tensor.matmul(
                        lg_ps[:, 512 * qc:512 * qc + 512],
                        kfull_sb[:, 128 * kt:128 * kt + 128],
                        QA[i][:, 512 * qc:512 * qc + 512])
                return lg_ps

            def emit_conv():
                conv_ps = psmisc.tile([64, 512], f32, tag="misc")
                nc.tensor.matmul(conv_ps[:, :], wconv_sb, xc_sb)
                conv_sb = projp.tile([64, 512], f32, tag="conv")
                nc.vector.tensor_copy(out=conv_sb, in_=conv_ps[:, :])
                nc.sync.dma_start(out=o_conv[:, :], in_=conv_sb)

            def emit_recip(i, half):
                r0 = 32 * i
                s = slice(512 * half, 512 * half + 512)
                with nc.allow_low_precision(reason="bf16 softmax denom"):
                    nc.vector.reciprocal(out=rp[r0:r0 + 1, s],
                                         in_=pv_sb[r0:r0 + 1, s])

            rpb = tailp.tile([128, HW], bf16, tag="rpb")

            def emit_bounce(i):
                r0 = 32 * i
                nc.gpsimd.dma_start(out=den_scr[i, :], in_=rp[r0:r0 + 1, :])
                rep = bass.AP(den_scr, i * HW, [[0, 32], [1, HW]])
                nc.gpsimd.dma_start(out=rpb[r0:r0 + 32, :], in_=rep)

            def emit_mul(i):
                r0 = 32 * i
                sl = slice(0, HW)
                nc.vector.tensor_mul(attn_n[r0:r0 + 32, sl],
                                     pv_sb[r0:r0 + 32, sl],
                                     rpb[r0:r0 + 32, sl])

            jobs = []
            jobs.extend(head_prep_chunks(1))
            next_prep = 2
            lg_tiles = {seq[0]: emit_lg(*seq[0]), seq[1]: emit_lg(*seq[1])}
            for j, (i, kt) in enumerate(seq):
                if kt == 7 and jobs:
                    jobs.pop(0)()
                if j + 2 < len(seq):
                    lg_tiles[seq[j + 2]] = emit_lg(*seq[j + 2])
                lg_ps = lg_tiles.pop((i, kt))
                pt = ptp.tile([128, HW], bf16)
                nc.scalar.activation(pt, lg_ps[:, :], AF.Exp)
                if debug and i == 0 and kt == 0:
                    nc.sync.dma_start(out=d_pt00[:, :], in_=pt)
                for qc in range(2):
                    nc.tensor.matmul(
                        pv_ps[32 * i:32 * i + 32, 512 * qc:512 * qc + 512],
                        vt_sb[:, kt, 32 * i:32 * i + 32],
                        pt[:, 512 * qc:512 * qc + 512],
                        start=(kt == 0), stop=(kt == 7),
                        tile_position=(0, 32 * i))
                if kt != 7 and jobs:
                    jobs.pop(0)()
                if kt == 7:
                    if i < 3:
                        nc.vector.tensor_copy(out=pv_sb[32 * i:32 * i + 32, :],
                                              in_=pv_ps[32 * i:32 * i + 32, :])
                        emit_recip(i, 0)
                        emit_recip(i, 1)
                        emit_bounce(i)
                    if next_prep < 4:
                        jobs.extend(head_prep_chunks(next_prep))
                        next_prep += 1
                    elif i == 2:
                        jobs.append(emit_conv)
                    if i < 3:
                        jobs.append(lambda i=i: emit_mul(i))
            if debug:
                nc.sync.dma_start(out=d_qa0[:, :], in_=QA[0])

            rpb = tailp.tile([128, HW], bf16, tag="rpb3", name="rpb3")
            oat_ps = pslg.tile([64, HW], f32, tag="lg")
            oat_sb = tailp.tile([64, HW], f32, tag="oat")
            HS = [slice(0, 512), slice(512, 1024)]
            recb3 = [None, None]
            with nc.allow_low_precision(reason="bf16 softmax denom"):
                for h, s in enumerate(HS):
                    nc.vector.reciprocal(out=rp[96:97, s], in_=pv_ps[96:97, s])
            for h, s in enumerate(HS):
                pool, tag = (psmisc, "misc") if h == 0 else (pslg, "lg")
                recb3[h] = pool.tile([128, HW], f32, tag=tag,
                                     name=f"recb3_{h}")
                nc.tensor.matmul(recb3[h][96:128, s], ones_sb[96:97, :],
                                 rp[96:97, s], tile_position=(96, 96))
                nc.scalar.activation(rpb[96:128, s], recb3[h][96:128, s],
                                     AF.Copy)
            for h, s in enumerate(HS):
                nc.vector.tensor_mul(attn_n[96:128, s], pv_ps[96:128, s],
                                     rpb[96:128, s])
            for h, s in enumerate(HS):
                nc.tensor.matmul(oat_ps[:, s], wattn_sb, attn_n[:, s])
            nc.vector.tensor_copy(out=oat_sb[:, HS[0]], in_=oat_ps[:, HS[0]])
            nc.sync.dma_start(out=o_attn[:, HS[0]], in_=oat_sb[:, HS[0]])
            nc.vector.tensor_copy(out=oat_sb[:, HS[1]], in_=oat_ps[:, HS[1]])
            nc.scalar.dma_start(out=o_attn[:, HS[1]], in_=oat_sb[:, HS[1]])

    nc.compile()
    return nc


def _host_prep(inputs):
    import ml_dtypes
    bf = ml_dtypes.bfloat16
    x = np.ascontiguousarray(inputs['x'], np.float32)
    w_qkv = np.ascontiguousarray(inputs['w_qkv'].reshape(2 * DK + DV, C), np.float32)
    b_qkv = np.ascontiguousarray(inputs['b_qkv'], np.float32)
    w_conv = np.ascontiguousarray(inputs['w_conv'].reshape(FILTERS - DV, C), np.float32)
    b_conv = np.ascontiguousarray(inputs['b_conv'], np.float32)
    w_attn = np.ascontiguousarray(inputs['w_attn'].reshape(DV, DV), np.float32)
    b_attn = np.ascontiguousarray(inputs['b_attn'], np.float32)
    rel_h = np.ascontiguousarray(inputs['key_rel_h'], np.float32)
    rel_w = np.ascontiguousarray(inputs['key_rel_w'], np.float32)
    relcat = np.concatenate([rel_w, rel_h], 0)

    kk = np.arange(HW)
    DCmat = np.zeros((64, HW), np.float32)
    DCmat[:32] = (kk[None, :] % 32 == np.arange(32)[:, None])
    DCmat[32:] = (kk[None, :] // 32 == np.arange(32)[:, None])

    wconv_aug = np.ascontiguousarray(
        np.concatenate([w_conv, b_conv[:, None]], 1).T)

    in_maps = []
    for cidx in range(N_CORES):
        b, g = cidx // 2, cidx % 2
        heads = [4 * g + i for i in range(4)]
        x_aug = np.concatenate([x[b].reshape(C, HW),
                                np.ones((1, HW), np.float32)], 0)
        kfull_m = np.concatenate([x[b].reshape(C, HW), DCmat], 0)
        wva_m = np.zeros((65, 4, 32), np.float32)
        wpat_m = np.zeros((65, 4, 126), np.float32)
        wkq_m = np.zeros((65, 4, 64), np.float32)
        for i, h in enumerate(heads):
            wv = w_qkv[128 + 8 * h:128 + 8 * h + 8]
            bv = b_qkv[128 + 8 * h:128 + 8 * h + 8]
            wva_m[64, i, 0] = 1.0
            wva_m[:64, i, 1:9] = wv.T
            wva_m[64, i, 1:9] = bv
            wq_h = w_qkv[8 * h:8 * h + 8] * SCALE
            bq_h = b_qkv[8 * h:8 * h + 8] * SCALE
            wk_h = w_qkv[64 + 8 * h:64 + 8 * h + 8]
            wpat_m[:64, i, :] = (relcat @ wq_h).T
            wpat_m[64, i, :] = relcat @ bq_h
            wq_aug = np.concatenate([wq_h, bq_h[:, None]], 1)
            wkq_m[:, i, :] = (wk_h.T @ wq_aug).T
        wattn_aug = np.zeros((128, 96), np.float32)
        wattn_aug[[0, 32, 64, 96], 64:96] = 1.0
        for i, h in enumerate(heads):
            wattn_aug[32 * i + 1:32 * i + 9, 0:64] = w_attn[:, 8 * h:8 * h + 8].T
        if g == 0:
            wattn_aug[0, 0:64] += b_attn
        xw_m = np.concatenate(
            [x_aug, wkq_m.reshape(65, 256), wpat_m.reshape(65, 504),
             wva_m.reshape(65, 128), wconv_aug,
             x_aug[:, 512 * g:512 * g + 512]], 1)
        assert xw_m.shape[1] == XW_COLS
        in_maps.append({
            'xw': np.ascontiguousarray(xw_m.astype(bf)),
            'kfull': np.ascontiguousarray(kfull_m.astype(bf)),
            'wtail': np.ascontiguousarray(wattn_aug.astype(bf)),
        })
    return in_maps


_CACHED = {}


def kernel(**inputs):
    from concourse.bass_utils import run_bass_kernel_spmd
    if 'nc' not in _CACHED:
        _CACHED['nc'] = _build_bass()
    nc = _CACHED['nc']
    in_maps = _host_prep(inputs)
    res = run_bass_kernel_spmd(nc, in_maps, core_ids=list(range(N_CORES)))
    out = np.zeros((B, FILTERS, HW), np.float32)
    for c in range(N_CORES):
        b, g = c // 2, c % 2
        out[b, :64, 512 * g:512 * g + 512] = res.results[c]['o_conv']
        out[b, 64:] += res.results[c]['o_attn']
    return out.reshape(B, FILTERS, H, W)
